# revision 1
# baseline (speedup 1.0000x reference)
"""ConvSTFT Trainium2 kernel.

Computes reference:
    x_b1t = pad(x_bt, 768 both sides)
    X = conv(x_b1t, V_d1m, stride=256)  -> (B, 1026, 628) -> (B, 2, 513, 628)

Equivalently, per batch: Out[f, t] = sum_k V[f, k] * x_pad[256 t + k].

Strategy (data-parallel over batch, 8 cores x 8 batches):
  - k = 128 c + p (c in 0..7, p = SBUF partition). Then
    x_pad[256 t + 128 c + p] = G[p, 2 t + c] where G[p, j] = x_pad[128 j + p]
    is the 128-transposed signal. G is built on-device: fp32->bf16 cast DMA
    load of x in (j, p) row-blocks, then 128x128 TensorE transposes
    (identity matmul; the xbar DMA transpose hits a walrus sync-wait limit
    under Tile) with a DVE copy PSUM -> SBUF.
  - V rows 513 and 1025 (imag of DC / Nyquist bins) are exactly zero, so the
    1026-row DFT matrix reduces to 1024 rows = 8 perfect 128-row tiles. The
    host prepends V^T in [p][c][f] order (bf16); the two zero rows are
    restored on unshard.
  - Matmul: out[f_tile, t] += VT[:, c, f_tile].T @ G[:, 2t+c], accumulated
    over c in PSUM (fp32), t in chunks of 512/116. PSUM -> SBUF via DVE,
    one 2.5 MB output DMA per batch.
"""

import numpy as np
import ml_dtypes

B = 64
T_SAMP = 160000
WIN = 1024
HOP = 256
DIM = 513  # rfft bins
PAD = 768
T_FRAMES = 628
N_CORES = 8
BPC = B // N_CORES  # batches per core
NJ = 1280  # G columns (padded)
XP_LEN = NJ * 128  # 163840 padded signal length
NB = NJ // 128  # 10 column blocks of G
FT_TILES = 8  # output row tiles (1024 rows / 128)
KC = 8  # contraction chunks (1024 / 128)
T_CHUNKS = ((0, 512), (512, T_FRAMES - 512))

_CACHE = {}


def _build_program(n_batches=BPC):
    import concourse.bass as bass
    import concourse.mybir as mybir
    from concourse.tile import TileContext
    from concourse.bass import ts

    f32 = mybir.dt.float32
    bf16 = mybir.dt.bfloat16

    nc = bass.Bass()
    xp = nc.declare_dram_parameter("xp", [BPC, XP_LEN], f32, isOutput=False)
    vt = nc.declare_dram_parameter("vt", [128, KC, 1024], bf16, isOutput=False)
    y = nc.declare_dram_parameter("y", [BPC, 1024, T_FRAMES], bf16, isOutput=True)

    from concourse.masks import make_identity
    from concourse.tile import add_dep_helper

    # This walrus build accepts at most ONE sync-wait per instruction. Tile's
    # kernel-tail drain carries the whole outstanding global clock (~19 sems)
    # on a single Drain and fails codegen. We instead emit a per-engine
    # ladder of 1-wait nops covering every DMA completion sem (below), after
    # which the all-engine barrier guarantees quiescence — so the tail drain
    # itself can skip its global sem waits.
    def _patched_drain_and_barrier(self, tick_clock, wait_clock):
        self.nc.sync.drain()
        self.nc.all_engine_barrier()
        assert self.sems is not None
        popped = self.nc._tile_sem_poison_stack.pop()
        assert popped is self._sem_poison
        self.nc.clear_and_free_semaphores(list(self.sems.allocated().values()))
        self.nc.all_engine_barrier()

    TileContext._drain_and_barrier = _patched_drain_and_barrier

    with TileContext(nc) as tc:
        with (
            tc.tile_pool(name="cpool", bufs=1) as cpool,
            tc.tile_pool(name="xpool", bufs=n_batches) as xpool,
            tc.tile_pool(name="gpool", bufs=n_batches) as gpool,
            tc.tile_pool(name="opool", bufs=n_batches) as opool,
            tc.tile_pool(name="pspool", bufs=6, space="PSUM") as pspool,
            tc.tile_pool(name="ptpool", bufs=2, space="PSUM") as ptpool,
        ):
            # SWDGE for the weight load: keeps HWDGE DMA count at 8 (the 8
            # per-batch output stores) so no DMAHW sem lane is ever reused —
            # walrus allows only one sync-wait on DMA-queue instructions.
            vts = cpool.tile([128, KC, 1024], bf16, name="vts")
            nc.gpsimd.dma_start(out=vts[:], in_=vt[:])
            ident = cpool.tile([128, 128], bf16, name="ident")
            make_identity(nc, ident)

            for b in range(n_batches):
                # Load x_pad[b] as [j_local, block, p] with fp32->bf16 cast.
                # flat sample index = 16384*a + 128*j_local + p
                xnat = xpool.tile([128, NB, 128], bf16, name="xnat")
                nc.gpsimd.dma_start(
                    out=xnat[:],
                    in_=xp[b].rearrange("(a j p) -> j a p", j=128, p=128),
                )
                # Transpose each 128x128 block: G[p, 128a + j] = xnat[j, a, p]
                g = gpool.tile([128, NJ], bf16, name="g")
                for a in range(NB):
                    tps = ptpool.tile([128, 128], bf16, name="tps")
                    nc.tensor.transpose(tps[:], xnat[:, a, :], ident[:])
                    nc.vector.tensor_copy(out=g[:, ts(a, 128)], in_=tps[:])

                outsb = opool.tile([128, FT_TILES, T_FRAMES], bf16, name="outsb")
                for ft in range(FT_TILES):
                    for t0, n in T_CHUNKS:
                        ps = pspool.tile([128, 512], f32, name="ps")
                        for c in range(KC):
                            nc.tensor.matmul(
                                ps[:, :n],
                                vts[:, c, ts(ft, 128)],
                                g[:, 2 * t0 + c : 2 * t0 + c + 2 * n : 2],
                                start=(c == 0),
                                stop=(c == KC - 1),
                            )
                        nc.vector.tensor_copy(
                            out=outsb[:, ft, t0 : t0 + n], in_=ps[:, :n]
                        )
                nc.sync.dma_start(
                    out=y[b].rearrange("(ft p) t -> p ft t", p=128), in_=outsb[:]
                )
    return nc


def _prep_inputs(x_bt, V_d1m):
    x_bt = np.asarray(x_bt, dtype=np.float32)
    V = np.asarray(V_d1m, dtype=np.float32).reshape(2 * DIM, WIN)
    xp = np.zeros((B, XP_LEN), dtype=np.float32)
    xp[:, PAD : PAD + T_SAMP] = x_bt
    # Drop the two identically-zero rows (513: imag DC, 1025: imag Nyquist).
    vk = np.delete(V, (DIM, 2 * DIM - 1), axis=0)  # (1024, 1024) [f, k]
    # vt[p, c, f] = vk[f, 128 c + p]
    vt = np.ascontiguousarray(
        vk.T.reshape(KC, 128, WIN).transpose(1, 0, 2)
    ).astype(ml_dtypes.bfloat16)
    return xp, vt


def _assemble_output(ys):
    yy = np.concatenate(ys, axis=0)  # (64, 1024, 628)
    out = np.zeros((B, 2 * DIM, T_FRAMES), dtype=np.float32)
    out[:, :DIM] = yy[:, :DIM]
    out[:, DIM + 1 : 2 * DIM - 1] = yy[:, DIM:]
    return out.reshape(B, 2, DIM, T_FRAMES)


def kernel(x_bt, V_d1m):
    from concourse.bass_utils import run_bass_kernel_spmd

    nc = _CACHE.get("nc")
    if nc is None:
        nc = _build_program()
        _CACHE["nc"] = nc

    xp, vt = _prep_inputs(x_bt, V_d1m)
    in_maps = [
        {"xp": xp[i * BPC : (i + 1) * BPC], "vt": vt} for i in range(N_CORES)
    ]
    res = run_bass_kernel_spmd(nc, in_maps, core_ids=list(range(N_CORES)))
    return _assemble_output([r["y"] for r in res.results])



# revision 28
# speedup vs baseline: 1.1294x; 1.1294x over previous
"""ConvSTFT Trainium2 kernel, even/odd-fold edition.

Reference: Out[f, t] = sum_k V[f, k] * x_pad[256 t + k], V = rDFT rows x
periodic Hamming window, output (B, 2, 513, 628).

Algorithm (per frame of 1024 samples): the window w[k] is symmetric
(w[1024-k] = w[k]), cos rows are k-symmetric and sin rows k-antisymmetric
about k=512.  With E[k] = x[k] + x[1024-k] and O[k] = x[k] - x[1024-k]
(k = 1..512; the E[512] slot carries 2 x[512], matrix coeff halved),
every cos row contracts over E and every sin row over O -- HALVING the
TensorE contraction vs the direct matmul, with the window folded into
the matrices.  Two small terms move to host assembly: the w[0]*x[256t]
rank-1 term (all cos rows) and the single cos f=512 row.

Device structure (per core = 8 batches, data-parallel over batch):
  - Host preps, per batch, a packed [128, 2564] bf16 block: columns
    0:1282 hold g2[p, j] = x_pad[128 j + p + 1], columns 1282:2564 hold
    r2[p, j'] = x_pad[128 (j'+4) + 127 - p], so the E/O slot
    (kap, p) <-> k = 128 kap + p + 1 builds from stride-2 slices:
        E[p,kap,t] = g2[p, 2t+kap] + r2[p, 2t+3-kap]       (8 DVE ops)
    t runs over 640 = 5 t-tiles of 128 (columns past t=627 come from the
    zero padding and are dropped on host).
  - Matmuls use the DATA as the stationary operand: for t-tile tau and
    kap, lhsT = E[:, kap, 128 tau :+128], rhs = V [slot, kap, f] (512 f
    cols), accumulating kap = 0..3 into one PSUM bank -> out[t, f] fp32.
    40 matmuls/batch at FD=512.
  - ScalarE evacuates PSUM (bf16 cast) into a per-batch [128, 5121]
    buffer whose tail also carried the input block (dead after the E/O
    adds); one output DMA per batch.

Walrus accepts at most ONE sync-wait per instruction and this build's
redundant-wait elider is disabled, so the program keeps every
instruction's dependency set inside a single semaphore:
  - every pooled buffer is used exactly once (bufs = n_batches), so no
    tile-reuse dependencies exist;
  - the E/O adds are emitted with e[kap=0] LAST, so the first real
    matmul's single DVE wait covers all eight adds (later ones elide);
  - each PSUM accumulation group opens with a 1-column zero matmul
    (start=True, using the identically-zero sin f=512 matrix column) so
    the PSUM-buffer-reuse wait (ScalarE) and the data wait (DVE) land on
    different instructions;
  - two 1-element ScalarE fences pre-pay the input-DMA-queue and DVE
    waits that the PSUM evacuations would otherwise owe for aliasing the
    input region; the input block's last column (ub[:, 5120]) is never
    overwritten and doubles as the fence-1 read cell.
"""

import numpy as np
import ml_dtypes

B = 64
T_SAMP = 160000
WIN = 1024
HOP = 256
DIM = 513
PAD = 768
T_FRAMES = 628
N_CORES = 8
BPC = B // N_CORES
NJG = 1258  # g2 columns (j <= 2*627+3 = 1257)
NJR = 1258  # r2 columns, stored shifted by 4 (orig j = 4..1261)
GRW = NJG + NJR  # 2516
XP_LEN = 161536  # covers r2's max sample 128*1261+127
TT = 640  # e/o frame cols: 5 t-tiles of 128 (cols 628+ stay uninit)
NTAU = 5
KC = 4  # contraction kap tiles (512 / 128)
YW = NTAU * 1024  # 5120 output cols per batch

_CACHE = {}


def _build_program(n_batches=BPC):
    import concourse.bass as bass
    import concourse.mybir as mybir
    from concourse.tile import TileContext, add_dep_helper

    f32 = mybir.dt.float32
    bf16 = mybir.dt.bfloat16

    nc = bass.Bass()
    xgr = nc.declare_dram_parameter("xgr", [BPC, 128, GRW], bf16, isOutput=False)
    vmat = nc.declare_dram_parameter(
        "vmat", [2, 128, KC, 512], bf16, isOutput=False
    )
    y = nc.declare_dram_parameter("y", [BPC, 128, YW], bf16, isOutput=True)

    # Tile's kernel-tail drain carries the whole outstanding global clock
    # on one Drain instruction and fails codegen under the one-wait
    # limit; replace with drain + all-engine barriers.
    def _patched_drain_and_barrier(self, tick_clock, wait_clock):
        self.nc.sync.drain()
        self.nc.all_engine_barrier()
        assert self.sems is not None
        popped = self.nc._tile_sem_poison_stack.pop()
        assert popped is self._sem_poison
        self.nc.clear_and_free_semaphores(list(self.sems.allocated().values()))
        self.nc.all_engine_barrier()

    TileContext._drain_and_barrier = _patched_drain_and_barrier

    with TileContext(nc) as tc:
        with (
            tc.tile_pool(name="cpool", bufs=1) as cpool,
            tc.tile_pool(name="ubpool", bufs=n_batches) as ubpool,
            tc.tile_pool(name="grpool", bufs=n_batches) as grpool,
            tc.tile_pool(name="epool", bufs=n_batches) as epool,
            tc.tile_pool(name="opool", bufs=n_batches) as opool,
            tc.tile_pool(name="pspool", bufs=4, space="PSUM") as pspool,
            tc.tile_pool(name="psfence", bufs=1, space="PSUM") as psfence,
        ):
            # SWDGE load keeps the 8 HWDGE queues exclusively for the
            # per-batch output stores.
            vms = cpool.tile([128, 2, KC, 512], bf16, name="vms")
            nc.gpsimd.dma_start(
                out=vms[:], in_=vmat.rearrange("m p k f -> p m k f")
            )
            # identically-zero column (sin f=512 row)
            zcol = vms[:, 1, 0, 511:512]

            # PE fence bank: per-batch fences write rotating disjoint cols
            # with start=False (never a bank clear) so no WAW dep forms;
            # the bank's garbage contents are never read.
            psf = psfence.tile([128, 512], f32, name="psf")
            # PE warmup: pays the vmat-queue wait on PE before any real
            # matmul needs it.
            nc.tensor.matmul(
                psf[0:1, 508:509], zcol, zcol,
                start=False, stop=True, skip_group_check=True,
            )

            for b in range(n_batches):
                gr = grpool.tile([128, GRW], bf16, name="gr")
                nc.gpsimd.dma_start(out=gr[:], in_=xgr[b])

                e = epool.tile([128, KC, TT], bf16, name="e")
                o = opool.tile([128, KC, TT], bf16, name="o")
                # o's first (carries the input-DMA-queue wait), e[kap=0]
                # LAST so a single DVE wait value covers every add.  Only
                # t < 628 is computed; e/o cols 628:640 stay uninitialized
                # (tau=4 matmuls read them into output rows the host
                # drops).
                nf = 2 * T_FRAMES - 1
                for kap in range(KC):
                    ga = gr[:, kap : kap + nf : 2]
                    rb = gr[:, NJG + 3 - kap : NJG + 3 - kap + nf : 2]
                    nc.vector.tensor_sub(o[:, kap, 0:T_FRAMES], ga, rb)
                for kap in range(KC - 1, -1, -1):
                    ga = gr[:, kap : kap + nf : 2]
                    rb = gr[:, NJG + 3 - kap : NJG + 3 - kap + nf : 2]
                    nc.vector.tensor_add(e[:, kap, 0:T_FRAMES], ga, rb)

                # PE fence: touches e and o (incl. the last DVE write,
                # e[kap=0]) so every real matmul's DVE wait elides; writes
                # rotating disjoint cols of the fence bank, start=False.
                nc.tensor.matmul(
                    psf[0:4, 4 * b : 4 * b + 4],
                    e[:, :, 0:1],
                    o[:, :, 0:1],
                    start=False,
                    stop=True,
                    skip_group_check=True,
                )

                ub = ubpool.tile([128, YW], bf16, name="ub")
                for tau in range(NTAU):
                    tsl = slice(128 * tau, 128 * tau + 128)
                    for half, dat in enumerate((e, o)):
                        ps = pspool.tile([128, 512], f32, name="ps")
                        for kap in range(KC):
                            nc.tensor.matmul(
                                ps[:],
                                dat[:, kap, tsl],
                                vms[:, half, kap, :],
                                start=(kap == 0),
                                stop=(kap == KC - 1),
                            )
                        base = 1024 * tau + 512 * half
                        nc.scalar.copy(
                            out=ub[:, base : base + 512], in_=ps[:]
                        )
                nc.sync.dma_start(out=y[b], in_=ub[:])
    return nc


def _make_mats():
    k = np.arange(1, 513, dtype=np.float64)
    n = np.arange(WIN)
    w = 0.54 - 0.46 * np.cos(2.0 * np.pi * n / WIN)
    f = np.arange(512, dtype=np.float64)
    vcos = w[1:513][None, :] * np.cos(
        2 * np.pi * f[:, None] * k[None, :] / WIN
    )  # [f, k-1]
    vcos[:, 511] *= 0.5  # E slot k=512 holds 2*x[512]
    fs_ = np.arange(1, 513, dtype=np.float64)
    vsin = -w[1:513][None, :] * np.sin(
        2 * np.pi * fs_[:, None] * k[None, :] / WIN
    )
    # device layout vmat[m, p, kap, f]: slot (kap, p) <-> k = 128 kap + p + 1
    vm = np.empty((2, 128, KC, 512), dtype=ml_dtypes.bfloat16)
    vm[0] = vcos.T.reshape(KC, 128, 512).transpose(1, 0, 2)
    vm[1] = vsin.T.reshape(KC, 128, 512).transpose(1, 0, 2)
    return vm, w


def _prep_inputs(x_bt):
    x_bt = np.asarray(x_bt, dtype=np.float32)
    xp = np.zeros((B, XP_LEN), dtype=np.float32)
    xp[:, PAD : PAD + T_SAMP] = x_bt
    gr = np.empty((B, 128, GRW), dtype=ml_dtypes.bfloat16)
    # g2[p, j] = xp[128 j + p + 1], j = 0..NJG-1
    gr[:, :, :NJG] = (
        xp[:, 1 : 1 + 128 * NJG].reshape(B, NJG, 128).transpose(0, 2, 1)
    )
    # r2[p, j'] = xp[128 (j'+4) + 127 - p], j' = 0..NJR-1
    gr[:, :, NJG:] = (
        xp[:, 128 * 4 : 128 * (4 + NJR)]
        .reshape(B, NJR, 128)[:, :, ::-1]
        .transpose(0, 2, 1)
    )
    return xp, gr


def _host_terms(xp, w):
    """Rank-1 w[0]*x[256t] term (for cos rows f=0..511) and cos f=512 row."""
    tidx = 256 * np.arange(T_FRAMES)
    x0 = xp[:, tidx].astype(np.float32)  # [B, T]
    sw = np.lib.stride_tricks.sliding_window_view(xp, WIN, axis=1)
    frames = sw[:, ::HOP][:, :T_FRAMES]  # [B, T, 1024]
    coeff = ((-1.0) ** np.arange(WIN) * w).astype(np.float32)
    cos512 = frames @ coeff  # [B, T]
    return x0, cos512


def _assemble_output(ys, x0, cos512, w0):
    yy = np.concatenate(ys, axis=0)  # [B, 128, 5120] bf16
    yy = np.asarray(yy, dtype=np.float32)
    yy = yy.reshape(B, 128, NTAU, 1024).transpose(0, 2, 1, 3)
    yy = yy.reshape(B, NTAU * 128, 1024)[:, :T_FRAMES]  # [B, T, 1024]
    out = np.zeros((B, 2, DIM, T_FRAMES), dtype=np.float32)
    out[:, 0, :512, :] = yy[:, :, :512].transpose(0, 2, 1) + (
        w0 * x0[:, None, :]
    )
    out[:, 0, 512, :] = cos512  # full row incl its k=0 term
    out[:, 1, 1:512, :] = yy[:, :, 512:1023].transpose(0, 2, 1)
    return out


def _make_in_maps(x_bt, V_d1m=None):
    xp, gr = _prep_inputs(x_bt)
    vm, _ = _make_mats()
    return [
        {"xgr": gr[i * BPC : (i + 1) * BPC], "vmat": vm}
        for i in range(N_CORES)
    ]


def kernel(x_bt, V_d1m):
    from concourse.bass_utils import run_bass_kernel_spmd

    nc = _CACHE.get("nc")
    if nc is None:
        nc = _build_program()
        _CACHE["nc"] = nc

    xp, gr = _prep_inputs(x_bt)
    vm, w = _make_mats()
    x0, cos512 = _host_terms(xp, w)
    in_maps = [
        {"xgr": gr[i * BPC : (i + 1) * BPC], "vmat": vm}
        for i in range(N_CORES)
    ]
    res = run_bass_kernel_spmd(nc, in_maps, core_ids=list(range(N_CORES)))
    return _assemble_output(
        [r["y"] for r in res.results], x0, cos512, float(w[0])
    )


# revision 43
# speedup vs baseline: 47.0497x; 41.6586x over previous
"""ConvSTFT Trainium2 kernel, even/odd-fold edition.

Reference: Out[f, t] = sum_k V[f, k] * x_pad[256 t + k], V = rDFT rows x
periodic Hamming window, output (B, 2, 513, 628).

Algorithm (per frame of 1024 samples): the window w[k] is symmetric
(w[1024-k] = w[k]), cos rows are k-symmetric and sin rows k-antisymmetric
about k=512.  With E[k] = x[k] + x[1024-k] and O[k] = x[k] - x[1024-k]
(k = 1..512; the E[512] slot carries 2 x[512], matrix coeff halved),
every cos row contracts over E and every sin row over O -- HALVING the
TensorE contraction vs the direct matmul, with the window folded into
the matrices.  Two small terms move to host assembly: the w[0]*x[256t]
rank-1 term (all cos rows) and the single cos f=512 row.

Device structure (per core = 8 batches, data-parallel over batch):
  - Host preps, per batch, a packed [128, 2564] bf16 block: columns
    0:1282 hold g2[p, j] = x_pad[128 j + p + 1], columns 1282:2564 hold
    r2[p, j'] = x_pad[128 (j'+4) + 127 - p], so the E/O slot
    (kap, p) <-> k = 128 kap + p + 1 builds from stride-2 slices:
        E[p,kap,t] = g2[p, 2t+kap] + r2[p, 2t+3-kap]       (8 DVE ops)
    t runs over 640 = 5 t-tiles of 128 (columns past t=627 come from the
    zero padding and are dropped on host).
  - Matmuls use the DATA as the stationary operand: for t-tile tau and
    kap, lhsT = E[:, kap, 128 tau :+128], rhs = V [slot, kap, f] (512 f
    cols), accumulating kap = 0..3 into one PSUM bank -> out[t, f] fp32.
    40 matmuls/batch at FD=512.
  - ScalarE evacuates PSUM (bf16 cast) into a per-batch [128, 5121]
    buffer whose tail also carried the input block (dead after the E/O
    adds); one output DMA per batch.

Walrus accepts at most ONE sync-wait per instruction and this build's
redundant-wait elider is disabled, so the program keeps every
instruction's dependency set inside a single semaphore:
  - every pooled buffer is used exactly once (bufs = n_batches), so no
    tile-reuse dependencies exist;
  - the E/O adds are emitted with e[kap=0] LAST, so the first real
    matmul's single DVE wait covers all eight adds (later ones elide);
  - each PSUM accumulation group opens with a 1-column zero matmul
    (start=True, using the identically-zero sin f=512 matrix column) so
    the PSUM-buffer-reuse wait (ScalarE) and the data wait (DVE) land on
    different instructions;
  - two 1-element ScalarE fences pre-pay the input-DMA-queue and DVE
    waits that the PSUM evacuations would otherwise owe for aliasing the
    input region; the input block's last column (ub[:, 5120]) is never
    overwritten and doubles as the fence-1 read cell.
"""

import numpy as np
import ml_dtypes

B = 64
T_SAMP = 160000
WIN = 1024
HOP = 256
DIM = 513
PAD = 768
T_FRAMES = 628
N_CORES = 8
BPC = B // N_CORES
NJG = 1258  # g2 columns (j <= 2*627+3 = 1257)
NJR = 1258  # r2 columns, stored shifted by 4 (orig j = 4..1261)
GRW = NJG + NJR  # 2516
XP_LEN = 161536  # covers r2's max sample 128*1261+127
TT = 640  # e/o frame cols: 5 t-tiles of 128 (cols 628+ stay uninit)
NTAU = 5
KC = 4  # contraction kap tiles (512 / 128)
YW = NTAU * 1024  # 5120 output cols per batch

_CACHE = {}


def _build_program(n_batches=BPC):
    import concourse.bass as bass
    import concourse.mybir as mybir
    from concourse.tile import TileContext, add_dep_helper

    f32 = mybir.dt.float32
    bf16 = mybir.dt.bfloat16

    nc = bass.Bass()
    xgr = nc.declare_dram_parameter("xgr", [BPC, 128, GRW], bf16, isOutput=False)
    vmat = nc.declare_dram_parameter(
        "vmat", [128, 2, KC, 512], bf16, isOutput=False
    )
    y = nc.declare_dram_parameter("y", [BPC, 128, YW], bf16, isOutput=True)

    # Tile's kernel-tail drain carries the whole outstanding global clock
    # on one Drain instruction and fails codegen under the one-wait
    # limit; replace with drain + all-engine barriers.
    def _patched_drain_and_barrier(self, tick_clock, wait_clock):
        self.nc.sync.drain()
        self.nc.all_engine_barrier()
        assert self.sems is not None
        popped = self.nc._tile_sem_poison_stack.pop()
        assert popped is self._sem_poison
        self.nc.clear_and_free_semaphores(list(self.sems.allocated().values()))
        self.nc.all_engine_barrier()

    TileContext._drain_and_barrier = _patched_drain_and_barrier

    with TileContext(nc) as tc:
        with (
            tc.tile_pool(name="cpool", bufs=1) as cpool,
            tc.tile_pool(name="ubpool", bufs=n_batches) as ubpool,
            tc.tile_pool(name="epool", bufs=n_batches) as epool,
            tc.tile_pool(name="opool", bufs=n_batches) as opool,
            tc.tile_pool(name="pspool", bufs=4, space="PSUM") as pspool,
            tc.tile_pool(name="psfence", bufs=1, space="PSUM") as psfence,
        ):
            # SWDGE loads keep the 8 HWDGE queues exclusively for the
            # per-batch output stores.  Inputs come in 3 DMAs (batch 0,
            # batch 1, batches 2..7) into one resident tile: few enough
            # that the ~1.2us/DMA SWDGE descriptor generation doesn't
            # serialize the kernel head, while batch 0's slice still
            # lands early.
            vms = cpool.tile([128, 2, KC, 512], bf16, name="vms")
            grall = cpool.tile([128, n_batches, GRW], bf16, name="grall")
            # batch 0's block first: the single SWDGE context serializes
            # transfers, and batch 0 gates the pipeline head
            nc.gpsimd.dma_start(
                out=grall[:, 0:1, :],
                in_=xgr[0:1].rearrange("b p j -> p b j"),
            )
            nc.gpsimd.dma_start(out=vms[:], in_=vmat[:])
            nc.gpsimd.dma_start(
                out=grall[:, 1:2, :],
                in_=xgr[1:2].rearrange("b p j -> p b j"),
            )
            nc.gpsimd.dma_start(
                out=grall[:, 2:n_batches, :],
                in_=xgr[2:n_batches].rearrange("b p j -> p b j"),
            )
            # identically-zero column (sin f=512 row)
            zcol = vms[:, 1, 0, 511:512]

            # PE fence bank: per-batch fences write rotating disjoint cols
            # with start=False (never a bank clear) so no WAW dep forms;
            # the bank's garbage contents are never read.
            psf = psfence.tile([128, 512], f32, name="psf")
            # PE warmup: pays the vmat-queue wait on PE before any real
            # matmul needs it.
            nc.tensor.matmul(
                psf[0:1, 508:509], zcol, zcol,
                start=False, stop=True, skip_group_check=True,
            )

            def eo_adds(gr, e, o, t0, nt):
                # o's first (a batch's first add carries its input-DMA-
                # queue wait), e[kap=0] LAST so the PE fence's single DVE
                # wait value covers every add of the span.  Only t < 628
                # is computed; e/o cols 628:640 stay uninitialized (tau=4
                # matmuls read them into output rows the host drops).
                nt = min(T_FRAMES, t0 + nt) - t0
                nf = 2 * nt - 1
                for kap in range(KC):
                    ga = gr[:, 2 * t0 + kap : 2 * t0 + kap + nf : 2]
                    rb = gr[
                        :, NJG + 2 * t0 + 3 - kap : NJG + 2 * t0 + 3 - kap + nf : 2
                    ]
                    nc.vector.tensor_sub(o[:, kap, t0 : t0 + nt], ga, rb)
                for kap in range(KC - 1, -1, -1):
                    ga = gr[:, 2 * t0 + kap : 2 * t0 + kap + nf : 2]
                    rb = gr[
                        :, NJG + 2 * t0 + 3 - kap : NJG + 2 * t0 + 3 - kap + nf : 2
                    ]
                    nc.vector.tensor_add(e[:, kap, t0 : t0 + nt], ga, rb)

            nfence = 0

            def pe_fence(e, o, t0):
                # PE fence (start=False, rotating disjoint cols of a bank
                # whose garbage contents are never read): one DVE wait
                # covering the adds of this span, so the real matmuls' DVE
                # waits elide.
                nonlocal nfence
                nc.tensor.matmul(
                    psf[0:4, 4 * nfence : 4 * nfence + 4],
                    e[:, :, t0 : t0 + 1],
                    o[:, :, t0 : t0 + 1],
                    start=False,
                    stop=True,
                    skip_group_check=True,
                )
                nfence += 1

            def mm_groups(e, o, ub, tau):
                tsl = slice(128 * tau, 128 * tau + 128)
                for half, dat in enumerate((e, o)):
                    ps = pspool.tile([128, 512], f32, name="ps")
                    for kap in range(KC):
                        nc.tensor.matmul(
                            ps[:],
                            dat[:, kap, tsl],
                            vms[:, half, kap, :],
                            start=(kap == 0),
                            stop=(kap == KC - 1),
                        )
                    base = 1024 * tau + 512 * half
                    nc.scalar.copy(out=ub[:, base : base + 512], in_=ps[:])

            for b in range(n_batches):
                gr = grall[:, b, :]
                e = epool.tile([128, KC, TT], bf16, name="e")
                o = opool.tile([128, KC, TT], bf16, name="o")
                ub = ubpool.tile([128, YW], bf16, name="ub")
                if b == 0:
                    # per-t-tile adds + fences so the first matmuls start
                    # as soon as one tile's E/O is ready
                    for tau in range(NTAU):
                        eo_adds(gr, e, o, 128 * tau, 128)
                        pe_fence(e, o, 128 * tau)
                        mm_groups(e, o, ub, tau)
                else:
                    eo_adds(gr, e, o, 0, TT)
                    pe_fence(e, o, 0)
                    for tau in range(NTAU):
                        mm_groups(e, o, ub, tau)
                nc.sync.dma_start(out=y[b], in_=ub[:])
    return nc


def _make_mats():
    k = np.arange(1, 513, dtype=np.float64)
    n = np.arange(WIN)
    w = 0.54 - 0.46 * np.cos(2.0 * np.pi * n / WIN)
    f = np.arange(512, dtype=np.float64)
    vcos = w[1:513][None, :] * np.cos(
        2 * np.pi * f[:, None] * k[None, :] / WIN
    )  # [f, k-1]
    vcos[:, 511] *= 0.5  # E slot k=512 holds 2*x[512]
    fs_ = np.arange(1, 513, dtype=np.float64)
    vsin = -w[1:513][None, :] * np.sin(
        2 * np.pi * fs_[:, None] * k[None, :] / WIN
    )
    # device layout vmat[p, m, kap, f]: slot (kap, p) <-> k = 128 kap + p + 1
    vm = np.empty((128, 2, KC, 512), dtype=ml_dtypes.bfloat16)
    vm[:, 0] = vcos.T.reshape(KC, 128, 512).transpose(1, 0, 2)
    vm[:, 1] = vsin.T.reshape(KC, 128, 512).transpose(1, 0, 2)
    return vm, w


def _prep_inputs(x_bt):
    x_bt = np.asarray(x_bt, dtype=np.float32)
    xp = np.zeros((B, XP_LEN), dtype=np.float32)
    xp[:, PAD : PAD + T_SAMP] = x_bt
    gr = np.empty((B, 128, GRW), dtype=ml_dtypes.bfloat16)
    # g2[p, j] = xp[128 j + p + 1], j = 0..NJG-1
    gr[:, :, :NJG] = (
        xp[:, 1 : 1 + 128 * NJG].reshape(B, NJG, 128).transpose(0, 2, 1)
    )
    # r2[p, j'] = xp[128 (j'+4) + 127 - p], j' = 0..NJR-1
    gr[:, :, NJG:] = (
        xp[:, 128 * 4 : 128 * (4 + NJR)]
        .reshape(B, NJR, 128)[:, :, ::-1]
        .transpose(0, 2, 1)
    )
    return xp, gr


def _host_terms(xp, w):
    """Rank-1 w[0]*x[256t] term (for cos rows f=0..511) and cos f=512 row."""
    tidx = 256 * np.arange(T_FRAMES)
    x0 = xp[:, tidx].astype(np.float32)  # [B, T]
    sw = np.lib.stride_tricks.sliding_window_view(xp, WIN, axis=1)
    frames = sw[:, ::HOP][:, :T_FRAMES]  # [B, T, 1024]
    coeff = ((-1.0) ** np.arange(WIN) * w).astype(np.float32)
    cos512 = frames @ coeff  # [B, T]
    return x0, cos512


def _assemble_output(ys, x0, cos512, w0):
    yy = np.concatenate(ys, axis=0)  # [B, 128, 5120] bf16
    yy = np.asarray(yy, dtype=np.float32)
    yy = yy.reshape(B, 128, NTAU, 1024).transpose(0, 2, 1, 3)
    yy = yy.reshape(B, NTAU * 128, 1024)[:, :T_FRAMES]  # [B, T, 1024]
    out = np.zeros((B, 2, DIM, T_FRAMES), dtype=np.float32)
    out[:, 0, :512, :] = yy[:, :, :512].transpose(0, 2, 1) + (
        w0 * x0[:, None, :]
    )
    out[:, 0, 512, :] = cos512  # full row incl its k=0 term
    out[:, 1, 1:512, :] = yy[:, :, 512:1023].transpose(0, 2, 1)
    return out


def _make_in_maps(x_bt, V_d1m=None):
    xp, gr = _prep_inputs(x_bt)
    vm, _ = _make_mats()
    return [
        {"xgr": gr[i * BPC : (i + 1) * BPC], "vmat": vm}
        for i in range(N_CORES)
    ]


def kernel(x_bt, V_d1m):
    from concourse.bass_utils import run_bass_kernel_spmd

    nc = _CACHE.get("nc")
    if nc is None:
        nc = _build_program()
        _CACHE["nc"] = nc

    xp, gr = _prep_inputs(x_bt)
    vm, w = _make_mats()
    x0, cos512 = _host_terms(xp, w)
    in_maps = [
        {"xgr": gr[i * BPC : (i + 1) * BPC], "vmat": vm}
        for i in range(N_CORES)
    ]
    res = run_bass_kernel_spmd(nc, in_maps, core_ids=list(range(N_CORES)))
    return _assemble_output(
        [r["y"] for r in res.results], x0, cos512, float(w[0])
    )
